# revision 43
# baseline (speedup 1.0000x reference)
"""Trainium2 Bass kernel for ActionVectorQuantizer (VQ codebook lookup).

reference:
    d = ||z||^2 + ||e||^2 - 2 z.e^T   -> idx = argmin_n d[..., n]
    z_q = z + stop_gradient(emb[idx] - z)      (forward value == emb[idx]
                                                up to ~1e-7 fp32 rounding)

Sharding: batch dim (8) -> one batch of [32768, 256] per NeuronCore.

Per-core algorithm (all heavy math on TensorE, DMA-bound overall):
  * host pre-splits z into bf16 hi/lo (z = zh + zl + O(2^-18)) and
    pre-transposes to [256, T] so the contraction dim (256) lands on
    SBUF partitions with plain contiguous DMAs. Total input bytes are
    unchanged (2 x bf16 == 1 x fp32).
  * scores s_n = z . e_n - 0.5||e_n||^2 via PSUM-accumulated bf16
    matmuls: (zh + zl) @ [eh|el] -> exact split product, error ~5e-6
    (well below both the ~0.66 typical top-2 gap and the reference's
    own ~3e-5 fp32 rounding noise).
  * argmax + first-index tie-break + exact one-hot on VectorE; the
    one-hot is written replicated x3 along the free dim so one PE
    transpose yields the stacked [24, 128] mask directly.
  * gather z_q = onehot @ emb via ONE K=24 matmul per 128-token group
    (bf16 3-way split of emb summed inside the array -> exact fp32).
  * 1-tile software pipeline skew so PE never waits on VectorE.
  * idx transposed on PE at the end, emitted as int32.
"""

import os

import numpy as np

N_CORES = 8
T = 32768  # tokens per core
D = 256
NCODE = 8
P = 128
G = 16  # token groups (of 128) per tile; multiple of 4
EMB_SPLITS = 3  # bf16 terms reconstructing emb exactly for the gather

_NC_CACHE = {}
LAST_RESULT = None


def _bf16_split(x, n):
    """Split float32 array x into n bf16 arrays summing to ~x."""
    import ml_dtypes

    out = []
    rem = x.astype(np.float32)
    for _ in range(n):
        h = rem.astype(ml_dtypes.bfloat16)
        out.append(h)
        rem = rem - h.astype(np.float32)
    return out


def build_nc(t_tokens=T, g=G):
    """Build the Bass kernel for one core processing [t_tokens, 256]."""
    import concourse.tile as tile
    import concourse.mybir as mybir
    from concourse import bacc
    from concourse.tile import add_dep_helper

    bf = mybir.dt.bfloat16
    f32 = mybir.dt.float32
    i32 = mybir.dt.int32
    AX = mybir.AxisListType
    OP = mybir.AluOpType

    S = EMB_SPLITS
    KG = 32  # gather contraction: S*8 stacked emb splits padded to 32
    ntiles = t_tokens // (P * g)
    ngrp = t_tokens // P  # total 128-token groups
    assert ntiles * P * g == t_tokens and g % 4 == 0

    nc = bacc.Bacc("TRN2", target_bir_lowering=False)

    zth_d = nc.dram_tensor("zth", [P, ntiles, 2, g * P], bf, kind="ExternalInput")
    ztl_d = nc.dram_tensor("ztl", [P, ntiles, 2, g * P], bf, kind="ExternalInput")
    rhsh_d = nc.dram_tensor("rhsh", [2 * P, 2 * NCODE], bf, kind="ExternalInput")
    # [128, 256]: 4 replicas (at base partitions 0/32/64/96) of the
    # [32, 256] stacked emb splits (rows 24:32 zero)
    embs_d = nc.dram_tensor("embs", [P, D], bf, kind="ExternalInput")
    bias_d = nc.dram_tensor("bias", [P, NCODE], f32, kind="ExternalInput")
    riota_d = nc.dram_tensor("riota", [P, NCODE], f32, kind="ExternalInput")
    idbf_d = nc.dram_tensor("idbf", [P, P], bf, kind="ExternalInput")
    idf_d = nc.dram_tensor("idf", [P, P], f32, kind="ExternalInput")

    zq_d = nc.dram_tensor("zq", [P, ngrp, D], f32, kind="ExternalOutput")
    idx_d = nc.dram_tensor("idx", [t_tokens], i32, kind="ExternalOutput")

    zth_v = zth_d.ap()
    ztl_v = ztl_d.ap()
    zq_v = zq_d.ap()

    with tile.TileContext(nc) as tc:
        with (
            tc.tile_pool(name="consts", bufs=1) as consts,
            tc.tile_pool(name="zin", bufs=4) as zin,
            tc.tile_pool(name="outp", bufs=3) as outp,
            tc.tile_pool(name="small", bufs=3) as small,
            tc.tile_pool(name="ohp", bufs=4) as ohp,
            tc.tile_pool(name="psum_s", bufs=3, space="PSUM") as psum_s_pool,
            tc.tile_pool(name="psum_oh", bufs=1, space="PSUM") as psum_oh_pool,
            tc.tile_pool(name="psum_zq", bufs=3, space="PSUM") as psum_zq_pool,
        ):
            # --- constants -------------------------------------------------
            rhsh_sb = consts.tile([P, 2, 2 * NCODE], bf)
            nc.sync.dma_start(rhsh_sb[:], rhsh_d.ap().rearrange("(c p) n -> p c n", p=P))
            embs_sb = consts.tile([P, D], bf)
            nc.sync.dma_start(embs_sb[:], embs_d.ap())
            bias_sb = consts.tile([P, NCODE], f32)
            nc.sync.dma_start(bias_sb[:], bias_d.ap())
            riota_sb = consts.tile([P, NCODE], f32)
            nc.sync.dma_start(riota_sb[:], riota_d.ap())
            idbf_sb = consts.tile([P, P], bf)
            nc.sync.dma_start(idbf_sb[:], idbf_d.ap())
            idf_sb = consts.tile([P, P], f32)
            nc.sync.dma_start(idf_sb[:], idf_d.ap())

            idxf_all = consts.tile([P, ngrp], f32)
            m_all = consts.tile([P, ngrp], f32)

            # --- software-pipelined main loop (SKEW-tile skew) ------------
            stage1 = {}  # n -> oh3 tile handle
            scores_last = {}  # n -> last score matmul instruction

            def front_half(n):
                """load + scores + argmax for tile n.

                Scores in orientation B: the 16-column [eh|el] is the
                stationary operand (cheap weight loads), z streams through
                the moving port in 512-token chunks -> scoresT [16, 512] in
                PSUM, copied to SBUF, then PE-transposed back to token-major
                [128, grp, 16]. cols 0:8 + cols 8:16 == (zh+zl).[eh|el].
                """
                zh_t = zin.tile([P, 2, g * P], bf, tag="zh")
                nc.scalar.dma_start(zh_t[:], zth_v[:, n])
                zl_t = zin.tile([P, 2, g * P], bf, tag="zl")
                nc.scalar.dma_start(zl_t[:], ztl_v[:, n])

                psum_s = psum_s_pool.tile([P, g, 2 * NCODE], f32)
                W = 512
                for w in range(g * P // W):
                    psum_sT = psum_zq_pool.tile([2 * NCODE, W], f32, tag="psum_zq")
                    sl = slice(w * W, (w + 1) * W)
                    k = 0
                    for zt in (zh_t, zl_t):
                        for c in range(2):
                            nc.tensor.matmul(
                                psum_sT[:],
                                rhsh_sb[:, c, :],
                                zt[:, c, sl],
                                start=(k == 0),
                                stop=(k == 3),
                            )
                            k += 1
                    sT_sb = small.tile([2 * NCODE, W], f32, tag="sT")
                    if w % 2 == 0:
                        nc.scalar.copy(sT_sb[:], psum_sT[:])
                    else:
                        nc.vector.tensor_copy(sT_sb[:], psum_sT[:])
                    for u in range(W // P):
                        grp = (w * W + u * P) // P
                        mm = nc.tensor.transpose(
                            psum_s[:, grp, :],
                            sT_sb[:, u * P : (u + 1) * P],
                            idf_sb[0 : 2 * NCODE, 0 : 2 * NCODE],
                        )
                scores_last[n] = mm

                # argmax + first-index tie-break + exact one-hot (VectorE)
                s_pre = small.tile([P, g, NCODE], f32, tag="s_pre")
                nc.vector.tensor_tensor(
                    s_pre[:],
                    psum_s[:, :, 0:NCODE],
                    bias_sb[:, None, :].to_broadcast([P, g, NCODE]),
                    op=OP.add,
                )
                s_sb = small.tile([P, g, NCODE], f32, tag="s_sb")
                nc.vector.tensor_tensor(
                    s_sb[:], s_pre[:], psum_s[:, :, NCODE:], op=OP.add
                )
                smax = small.tile([P, g], f32, tag="smax")
                nc.vector.tensor_reduce(smax[:], s_sb[:], axis=AX.X, op=OP.max)
                eq = small.tile([P, g, NCODE], f32, tag="eq")
                nc.vector.tensor_tensor(
                    eq[:],
                    s_sb[:],
                    smax[:, :, None].to_broadcast([P, g, NCODE]),
                    op=OP.is_equal,
                )
                pick = small.tile([P, g, NCODE], f32, tag="pick")
                nc.vector.tensor_tensor(
                    pick[:],
                    eq[:],
                    riota_sb[:, None, :].to_broadcast([P, g, NCODE]),
                    op=OP.mult,
                )
                m = m_all[:, n * g : (n + 1) * g]
                nc.vector.tensor_reduce(m, pick[:], axis=AX.X, op=OP.max)
                # one-hot, replicated S times along free dim -> transposes
                # directly into the stacked [24, 128] gather mask
                oh3 = ohp.tile([P, g, S, NCODE], bf, tag="oh3")
                nc.vector.tensor_tensor(
                    oh3[:],
                    pick[:, :, None, :].to_broadcast([P, g, S, NCODE]),
                    m[:, :, None, None].to_broadcast([P, g, S, NCODE]),
                    op=OP.is_equal,
                )
                stage1[n] = oh3

            def back_half(n):
                """transpose masks + gather + store for tile n."""
                oh3 = stage1.pop(n)
                psum_oh = psum_oh_pool.tile([S * NCODE, g, P], bf)
                for grp in range(g):
                    tr = nc.tensor.transpose(
                        psum_oh[:, grp, :],
                        oh3[:, grp, :, :].rearrange("p s n -> p (s n)"),
                        idbf_sb[:],
                    )
                    if grp == 0 and (n + SKEW) in scores_last:
                        # pipeline hint: keep PE on tile n+SKEW's scores while
                        # VectorE finishes tile n's argmax, then transpose
                        add_dep_helper(
                            tr.ins,
                            scores_last[n + SKEW].ins,
                            sync=False,
                            reason="sw-pipeline skew",
                        )
                maskt = small.tile([S * NCODE, g, P], bf, tag="maskt")
                nc.scalar.copy(maskt[:], psum_oh[:])

                out_sb = outp.tile([P, g, D], f32, tag="out")
                for j in range(g // 2):
                    psum_zq = psum_zq_pool.tile([P, 2, D], f32)
                    for kk in range(2):
                        grp = 2 * j + kk
                        nc.tensor.matmul(
                            psum_zq[:, kk, :],
                            maskt[:, grp, :],
                            embs_sb[0 : S * NCODE, :],
                            start=True,
                            stop=True,
                        )
                    dst = out_sb[:, 2 * j : 2 * j + 2, :]
                    if j % 2 == 0:
                        nc.scalar.copy(dst, psum_zq[:])
                    else:
                        nc.vector.tensor_copy(dst, psum_zq[:])

                nc.sync.dma_start(zq_v[:, n * g : (n + 1) * g, :], out_sb[:])

            SKEW = 2
            for n in range(ntiles + SKEW):
                if n < ntiles:
                    front_half(n)
                if n >= SKEW:
                    back_half(n - SKEW)

            # --- idx epilogue: idx = 8 - m, transpose, emit int32 ----------
            nc.vector.tensor_scalar(
                idxf_all[:], m_all[:], -1.0, float(NCODE), op0=OP.mult, op1=OP.add
            )
            nchunk = ngrp // P
            psum_idx = psum_zq_pool.tile([P, nchunk, P], f32, tag="psum_zq")
            for c in range(nchunk):
                nc.tensor.transpose(
                    psum_idx[:, c, :], idxf_all[:, c * P : (c + 1) * P], idf_sb[:]
                )
            idxt = consts.tile([P, nchunk, P], i32)
            nc.scalar.copy(idxt[:], psum_idx[:])
            nc.sync.dma_start(
                idx_d.ap().rearrange("(c g p) -> g c p", c=nchunk, p=P), idxt[:]
            )

    nc.compile()
    return nc


def _host_prep(z, emb):
    """Build per-core and shared input arrays (all numpy)."""
    import ml_dtypes  # noqa: F401

    z = np.asarray(z, dtype=np.float32)
    emb = np.asarray(emb, dtype=np.float32)
    b, t, d = z.shape
    assert d == D and emb.shape == (NCODE, D)

    # transpose then split; device layout [P, ntiles, 2, g*P]:
    # [p, n, c, t'] = zT[c*128 + p, n*(g*P) + t']
    zt = np.ascontiguousarray(z.transpose(0, 2, 1))
    zth, ztl = _bf16_split(zt, 2)
    ntiles = t // (P * G)

    def dev_layout(a):  # [B, 256, T] -> [B, P, ntiles, 2, G*P]
        return np.ascontiguousarray(
            a.reshape(b, 2, P, ntiles, G * P).transpose(0, 2, 3, 1, 4)
        )

    zth = dev_layout(zth)
    ztl = dev_layout(ztl)

    et = np.ascontiguousarray(emb.T)  # [256, 8]
    eh, el = _bf16_split(et, 2)
    rhsh = np.ascontiguousarray(np.concatenate([eh, el], axis=1))  # [256, 16]

    # stacked bf16 emb splits [24, 256], padded to 32 rows of zeros,
    # replicated 4x -> [128, 256] (mask row-groups at base 0/32/64/96)
    e24 = np.stack(_bf16_split(emb, EMB_SPLITS)).reshape(
        EMB_SPLITS * NCODE, D
    )
    e32 = np.zeros((32, D), dtype=e24.dtype)
    e32[: EMB_SPLITS * NCODE] = e24
    embs = np.ascontiguousarray(np.tile(e32, (4, 1)))  # [128, 256]

    bias = (-0.5 * np.sum(emb.astype(np.float64) ** 2, axis=1)).astype(np.float32)
    bias_rep = np.ascontiguousarray(np.tile(bias[None, :], (P, 1)))
    riota = np.ascontiguousarray(
        np.tile(np.arange(NCODE, 0, -1, dtype=np.float32)[None, :], (P, 1))
    )
    idbf = np.eye(P, dtype=ml_dtypes.bfloat16)
    idf = np.eye(P, dtype=np.float32)

    shared = {
        "rhsh": rhsh,
        "embs": embs,
        "bias": bias_rep,
        "riota": riota,
        "idbf": idbf,
        "idf": idf,
    }
    in_maps = []
    for i in range(b):
        m = dict(shared)
        m["zth"] = zth[i]
        m["ztl"] = ztl[i]
        in_maps.append(m)
    return in_maps


def _maybe_install_ntff_shim():
    """Best-effort: register the axon NTFF profile hook if absent."""
    import sys
    import types

    if "antenv.axon_hooks" in sys.modules:
        return
    try:
        import antenv

        mod = types.ModuleType("antenv.axon_hooks")
        mod._hook = None
        mod.set_axon_ntff_profile_hook = lambda h: setattr(mod, "_hook", h)
        mod.get_axon_ntff_profile_hook = lambda: mod._hook
        sys.modules["antenv.axon_hooks"] = mod
        antenv.axon_hooks = mod

        from trn_agent_boot.trn_boot import _ntff_profile_via_ctypes

        mod._hook = _ntff_profile_via_ctypes("/opt/axon/libaxon_pjrt.so")
    except Exception:
        pass


def kernel(z, emb):
    global LAST_RESULT
    import concourse.bass_utils as bass_utils

    z = np.asarray(z, dtype=np.float32)
    emb = np.asarray(emb, dtype=np.float32)
    in_maps = _host_prep(z, emb)

    key = (T, G)
    if key not in _NC_CACHE:
        _NC_CACHE[key] = build_nc(T, G)
    nc = _NC_CACHE[key]

    trace = bool(os.environ.get("VQ_TRACE"))
    if trace:
        _maybe_install_ntff_shim()
    res = bass_utils.run_bass_kernel_spmd(
        nc, in_maps, core_ids=list(range(N_CORES)), trace=trace
    )
    LAST_RESULT = res

    # device zq layout is [P, T//P, D] (token t = j*128 + p at [p, j, :])
    z_q = np.stack(
        [r["zq"].transpose(1, 0, 2).reshape(T, D) for r in res.results]
    ).astype(np.float32)
    idx = np.stack([r["idx"] for r in res.results]).astype(np.int32)
    return z_q, idx


# revision 44
# speedup vs baseline: 1.4666x; 1.4666x over previous
"""Trainium2 Bass kernel for ActionVectorQuantizer (VQ codebook lookup).

reference:
    d = ||z||^2 + ||e||^2 - 2 z.e^T   -> idx = argmin_n d[..., n]
    z_q = z + stop_gradient(emb[idx] - z)      (forward value == emb[idx]
                                                up to ~1e-7 fp32 rounding)

Sharding: batch dim (8) -> one batch of [32768, 256] per NeuronCore.

Per-core algorithm (all heavy math on TensorE, DMA-bound overall):
  * host pre-splits z into bf16 hi/lo (z = zh + zl + O(2^-18)) and
    pre-transposes to [256, T] so the contraction dim (256) lands on
    SBUF partitions with plain contiguous DMAs. Total input bytes are
    unchanged (2 x bf16 == 1 x fp32).
  * scores s_n = z . e_n - 0.5||e_n||^2 via PSUM-accumulated bf16
    matmuls: (zh + zl) @ [eh|el] -> exact split product, error ~5e-6
    (well below both the ~0.66 typical top-2 gap and the reference's
    own ~3e-5 fp32 rounding noise).
  * argmax + first-index tie-break + exact one-hot on VectorE; the
    one-hot is written replicated x3 along the free dim so one PE
    transpose yields the stacked [24, 128] mask directly.
  * gather z_q = onehot @ emb via ONE K=24 matmul per 128-token group
    (bf16 3-way split of emb summed inside the array -> exact fp32).
  * 1-tile software pipeline skew so PE never waits on VectorE.
  * idx transposed on PE at the end, emitted as int32.
"""

import os

import numpy as np

N_CORES = 8
T = 32768  # tokens per core
D = 256
NCODE = 8
P = 128
G = 16  # token groups (of 128) per tile; multiple of 4
EMB_SPLITS = 3  # bf16 terms reconstructing emb exactly for the gather

_NC_CACHE = {}
LAST_RESULT = None


def _bf16_split(x, n):
    """Split float32 array x into n bf16 arrays summing to ~x."""
    import ml_dtypes

    out = []
    rem = x.astype(np.float32)
    for _ in range(n):
        h = rem.astype(ml_dtypes.bfloat16)
        out.append(h)
        rem = rem - h.astype(np.float32)
    return out


def build_nc(t_tokens=T, g=G):
    """Build the Bass kernel for one core processing [t_tokens, 256]."""
    import concourse.tile as tile
    import concourse.mybir as mybir
    from concourse import bacc
    from concourse.tile import add_dep_helper

    bf = mybir.dt.bfloat16
    f32 = mybir.dt.float32
    i32 = mybir.dt.int32
    AX = mybir.AxisListType
    OP = mybir.AluOpType

    S = EMB_SPLITS
    KG = 32  # gather contraction: S*8 stacked emb splits padded to 32
    ntiles = t_tokens // (P * g)
    ngrp = t_tokens // P  # total 128-token groups
    assert ntiles * P * g == t_tokens and g % 4 == 0

    nc = bacc.Bacc("TRN2", target_bir_lowering=False)

    zth_d = nc.dram_tensor("zth", [P, ntiles, 2, g * P], bf, kind="ExternalInput")
    ztl_d = nc.dram_tensor("ztl", [P, ntiles, 2, g * P], bf, kind="ExternalInput")
    rhsh_d = nc.dram_tensor("rhsh", [2 * P, 2 * NCODE], bf, kind="ExternalInput")
    # [128, 256]: 4 replicas (at base partitions 0/32/64/96) of the
    # [32, 256] stacked emb splits (rows 24:32 zero)
    embs_d = nc.dram_tensor("embs", [P, D], bf, kind="ExternalInput")
    bias_d = nc.dram_tensor("bias", [P, NCODE], f32, kind="ExternalInput")
    riota_d = nc.dram_tensor("riota", [P, NCODE], f32, kind="ExternalInput")
    idbf_d = nc.dram_tensor("idbf", [P, P], bf, kind="ExternalInput")
    idf_d = nc.dram_tensor("idf", [P, P], f32, kind="ExternalInput")

    zq_d = nc.dram_tensor("zq", [P, ngrp, D], f32, kind="ExternalOutput")
    idx_d = nc.dram_tensor("idx", [t_tokens], i32, kind="ExternalOutput")

    zth_v = zth_d.ap()
    ztl_v = ztl_d.ap()
    zq_v = zq_d.ap()

    with tile.TileContext(nc) as tc:
        with (
            tc.tile_pool(name="consts", bufs=1) as consts,
            tc.tile_pool(name="zin", bufs=4) as zin,
            tc.tile_pool(name="outp", bufs=3) as outp,
            tc.tile_pool(name="small", bufs=3) as small,
            tc.tile_pool(name="ohp", bufs=4) as ohp,
            tc.tile_pool(name="psum_s", bufs=2, space="PSUM") as psum_s_pool,
            tc.tile_pool(name="psum_st", bufs=2, space="PSUM") as psum_st_pool,
            tc.tile_pool(name="psum_oh", bufs=1, space="PSUM") as psum_oh_pool,
            tc.tile_pool(name="psum_zq", bufs=2, space="PSUM") as psum_zq_pool,
        ):
            # --- constants -------------------------------------------------
            rhsh_sb = consts.tile([P, 2, 2 * NCODE], bf)
            nc.sync.dma_start(rhsh_sb[:], rhsh_d.ap().rearrange("(c p) n -> p c n", p=P))
            embs_sb = consts.tile([P, D], bf)
            nc.sync.dma_start(embs_sb[:], embs_d.ap())
            bias_sb = consts.tile([P, NCODE], f32)
            nc.sync.dma_start(bias_sb[:], bias_d.ap())
            riota_sb = consts.tile([P, NCODE], f32)
            nc.sync.dma_start(riota_sb[:], riota_d.ap())
            idbf_sb = consts.tile([P, P], bf)
            nc.sync.dma_start(idbf_sb[:], idbf_d.ap())
            idf_sb = consts.tile([P, P], f32)
            nc.sync.dma_start(idf_sb[:], idf_d.ap())

            idxf_all = consts.tile([P, ngrp], f32)
            m_all = consts.tile([P, ngrp], f32)

            # --- software-pipelined main loop (SKEW-tile skew) ------------
            stage1 = {}  # n -> oh3 tile handle
            scores_last = {}  # n -> last score matmul instruction

            def front_half(n):
                """load + scores + argmax for tile n.

                Scores in orientation B: the 16-column [eh|el] is the
                stationary operand (cheap weight loads), z streams through
                the moving port in 512-token chunks -> scoresT [16, 512] in
                PSUM, copied to SBUF, then PE-transposed back to token-major
                [128, grp, 16]. cols 0:8 + cols 8:16 == (zh+zl).[eh|el].
                """
                zh_t = zin.tile([P, 2, g * P], bf, tag="zh")
                nc.scalar.dma_start(zh_t[:], zth_v[:, n])
                zl_t = zin.tile([P, 2, g * P], bf, tag="zl")
                nc.scalar.dma_start(zl_t[:], ztl_v[:, n])

                psum_s = psum_s_pool.tile([P, g, 2 * NCODE], f32)
                W = 512
                for w in range(g * P // W):
                    psum_sT = psum_st_pool.tile([2 * NCODE, W], f32)
                    sl = slice(w * W, (w + 1) * W)
                    k = 0
                    for zt in (zh_t, zl_t):
                        for c in range(2):
                            nc.tensor.matmul(
                                psum_sT[:],
                                rhsh_sb[:, c, :],
                                zt[:, c, sl],
                                start=(k == 0),
                                stop=(k == 3),
                            )
                            k += 1
                    sT_sb = small.tile([2 * NCODE, W], f32, tag="sT")
                    nc.scalar.copy(sT_sb[:], psum_sT[:])
                    for u in range(W // P):
                        grp = (w * W + u * P) // P
                        mm = nc.tensor.transpose(
                            psum_s[:, grp, :],
                            sT_sb[:, u * P : (u + 1) * P],
                            idf_sb[0 : 2 * NCODE, 0 : 2 * NCODE],
                        )
                scores_last[n] = mm

                # argmax + first-index tie-break + exact one-hot (VectorE)
                s_pre = small.tile([P, g, NCODE], f32, tag="s_pre")
                nc.vector.tensor_tensor(
                    s_pre[:],
                    psum_s[:, :, 0:NCODE],
                    bias_sb[:, None, :].to_broadcast([P, g, NCODE]),
                    op=OP.add,
                )
                s_sb = small.tile([P, g, NCODE], f32, tag="s_sb")
                nc.vector.tensor_tensor(
                    s_sb[:], s_pre[:], psum_s[:, :, NCODE:], op=OP.add
                )
                smax = small.tile([P, g], f32, tag="smax")
                nc.vector.tensor_reduce(smax[:], s_sb[:], axis=AX.X, op=OP.max)
                eq = small.tile([P, g, NCODE], f32, tag="eq")
                nc.vector.tensor_tensor(
                    eq[:],
                    s_sb[:],
                    smax[:, :, None].to_broadcast([P, g, NCODE]),
                    op=OP.is_equal,
                )
                pick = small.tile([P, g, NCODE], f32, tag="pick")
                nc.vector.tensor_tensor(
                    pick[:],
                    eq[:],
                    riota_sb[:, None, :].to_broadcast([P, g, NCODE]),
                    op=OP.mult,
                )
                m = m_all[:, n * g : (n + 1) * g]
                nc.vector.tensor_reduce(m, pick[:], axis=AX.X, op=OP.max)
                # one-hot, replicated S times along free dim -> transposes
                # directly into the stacked [24, 128] gather mask
                oh3 = ohp.tile([P, g, S, NCODE], bf, tag="oh3")
                nc.vector.tensor_tensor(
                    oh3[:],
                    pick[:, :, None, :].to_broadcast([P, g, S, NCODE]),
                    m[:, :, None, None].to_broadcast([P, g, S, NCODE]),
                    op=OP.is_equal,
                )
                stage1[n] = oh3

            def back_half(n):
                """transpose masks + gather + store for tile n."""
                oh3 = stage1.pop(n)
                psum_oh = psum_oh_pool.tile([S * NCODE, g, P], bf)
                for grp in range(g):
                    tr = nc.tensor.transpose(
                        psum_oh[:, grp, :],
                        oh3[:, grp, :, :].rearrange("p s n -> p (s n)"),
                        idbf_sb[:],
                    )
                    if grp == 0 and (n + SKEW) in scores_last:
                        # pipeline hint: keep PE on tile n+SKEW's scores while
                        # VectorE finishes tile n's argmax, then transpose
                        add_dep_helper(
                            tr.ins,
                            scores_last[n + SKEW].ins,
                            sync=False,
                            reason="sw-pipeline skew",
                        )
                maskt = small.tile([S * NCODE, g, P], bf, tag="maskt")
                nc.scalar.copy(maskt[:], psum_oh[:])

                out_sb = outp.tile([P, g, D], f32, tag="out")
                for j in range(g // 2):
                    psum_zq = psum_zq_pool.tile([P, 2, D], f32)
                    for kk in range(2):
                        grp = 2 * j + kk
                        nc.tensor.matmul(
                            psum_zq[:, kk, :],
                            maskt[:, grp, :],
                            embs_sb[0 : S * NCODE, :],
                            start=True,
                            stop=True,
                        )
                    dst = out_sb[:, 2 * j : 2 * j + 2, :]
                    nc.vector.tensor_copy(dst, psum_zq[:])

                nc.sync.dma_start(zq_v[:, n * g : (n + 1) * g, :], out_sb[:])

            SKEW = 2
            for n in range(ntiles + SKEW):
                if n < ntiles:
                    front_half(n)
                if n >= SKEW:
                    back_half(n - SKEW)

            # --- idx epilogue: idx = 8 - m, transpose, emit int32 ----------
            nc.vector.tensor_scalar(
                idxf_all[:], m_all[:], -1.0, float(NCODE), op0=OP.mult, op1=OP.add
            )
            nchunk = ngrp // P
            psum_idx = psum_zq_pool.tile([P, nchunk, P], f32, tag="psum_zq")
            for c in range(nchunk):
                nc.tensor.transpose(
                    psum_idx[:, c, :], idxf_all[:, c * P : (c + 1) * P], idf_sb[:]
                )
            idxt = consts.tile([P, nchunk, P], i32)
            nc.scalar.copy(idxt[:], psum_idx[:])
            nc.sync.dma_start(
                idx_d.ap().rearrange("(c g p) -> g c p", c=nchunk, p=P), idxt[:]
            )

    nc.compile()
    return nc


def _host_prep(z, emb):
    """Build per-core and shared input arrays (all numpy)."""
    import ml_dtypes  # noqa: F401

    z = np.asarray(z, dtype=np.float32)
    emb = np.asarray(emb, dtype=np.float32)
    b, t, d = z.shape
    assert d == D and emb.shape == (NCODE, D)

    # transpose then split; device layout [P, ntiles, 2, g*P]:
    # [p, n, c, t'] = zT[c*128 + p, n*(g*P) + t']
    zt = np.ascontiguousarray(z.transpose(0, 2, 1))
    zth, ztl = _bf16_split(zt, 2)
    ntiles = t // (P * G)

    def dev_layout(a):  # [B, 256, T] -> [B, P, ntiles, 2, G*P]
        return np.ascontiguousarray(
            a.reshape(b, 2, P, ntiles, G * P).transpose(0, 2, 3, 1, 4)
        )

    zth = dev_layout(zth)
    ztl = dev_layout(ztl)

    et = np.ascontiguousarray(emb.T)  # [256, 8]
    eh, el = _bf16_split(et, 2)
    rhsh = np.ascontiguousarray(np.concatenate([eh, el], axis=1))  # [256, 16]

    # stacked bf16 emb splits [24, 256], padded to 32 rows of zeros,
    # replicated 4x -> [128, 256] (mask row-groups at base 0/32/64/96)
    e24 = np.stack(_bf16_split(emb, EMB_SPLITS)).reshape(
        EMB_SPLITS * NCODE, D
    )
    e32 = np.zeros((32, D), dtype=e24.dtype)
    e32[: EMB_SPLITS * NCODE] = e24
    embs = np.ascontiguousarray(np.tile(e32, (4, 1)))  # [128, 256]

    bias = (-0.5 * np.sum(emb.astype(np.float64) ** 2, axis=1)).astype(np.float32)
    bias_rep = np.ascontiguousarray(np.tile(bias[None, :], (P, 1)))
    riota = np.ascontiguousarray(
        np.tile(np.arange(NCODE, 0, -1, dtype=np.float32)[None, :], (P, 1))
    )
    idbf = np.eye(P, dtype=ml_dtypes.bfloat16)
    idf = np.eye(P, dtype=np.float32)

    shared = {
        "rhsh": rhsh,
        "embs": embs,
        "bias": bias_rep,
        "riota": riota,
        "idbf": idbf,
        "idf": idf,
    }
    in_maps = []
    for i in range(b):
        m = dict(shared)
        m["zth"] = zth[i]
        m["ztl"] = ztl[i]
        in_maps.append(m)
    return in_maps


def _maybe_install_ntff_shim():
    """Best-effort: register the axon NTFF profile hook if absent."""
    import sys
    import types

    if "antenv.axon_hooks" in sys.modules:
        return
    try:
        import antenv

        mod = types.ModuleType("antenv.axon_hooks")
        mod._hook = None
        mod.set_axon_ntff_profile_hook = lambda h: setattr(mod, "_hook", h)
        mod.get_axon_ntff_profile_hook = lambda: mod._hook
        sys.modules["antenv.axon_hooks"] = mod
        antenv.axon_hooks = mod

        from trn_agent_boot.trn_boot import _ntff_profile_via_ctypes

        mod._hook = _ntff_profile_via_ctypes("/opt/axon/libaxon_pjrt.so")
    except Exception:
        pass


def kernel(z, emb):
    global LAST_RESULT
    import concourse.bass_utils as bass_utils

    z = np.asarray(z, dtype=np.float32)
    emb = np.asarray(emb, dtype=np.float32)
    in_maps = _host_prep(z, emb)

    key = (T, G)
    if key not in _NC_CACHE:
        _NC_CACHE[key] = build_nc(T, G)
    nc = _NC_CACHE[key]

    trace = bool(os.environ.get("VQ_TRACE"))
    if trace:
        _maybe_install_ntff_shim()
    res = bass_utils.run_bass_kernel_spmd(
        nc, in_maps, core_ids=list(range(N_CORES)), trace=trace
    )
    LAST_RESULT = res

    # device zq layout is [P, T//P, D] (token t = j*128 + p at [p, j, :])
    z_q = np.stack(
        [r["zq"].transpose(1, 0, 2).reshape(T, D) for r in res.results]
    ).astype(np.float32)
    idx = np.stack([r["idx"] for r in res.results]).astype(np.int32)
    return z_q, idx
